# revision 1
# baseline (speedup 1.0000x reference)
"""Locally-connected conv (per-location weights) + ReLU on 8 Trainium2 cores.

Problem: x (B=64, Cin=64, H=64, W=64), weights (H, W, Cout=64, Cin=64, 3, 3)
  out[r,a,i,j] = relu( sum_{b,c,d} weights[i,j,a,b,c,d] * xpad[r,b,i+c,j+d] )

Sharding: data-parallel over H — core cid owns output rows i in [8*cid, 8*cid+8).
No collectives; pure SPMD with per-core input slices.

Device strategy (per core):
  - Host pre-packs weights into contraction-major tiles so every DMA has
    multi-KB contiguous partition lines (full HBM bandwidth).
  - x is padded/transposed on host to x_t[b, u, r, v] (u=h+1, v=w+1 padded
    planes); pairs of planes are stacked into 128-partition SBUF tiles so a
    single K=128 matmul contracts Cin x 2 vertical taps at once.
  - Per output row-pair and 16-column block: 3 dual-tap (K=128) + pairs of
    single-tap (K=64, opposite partition halves, run concurrently on the PE
    via row-group tiling) matmuls per location accumulate into PSUM.
  - One PSUM bank holds 8 locations; a single start=True on the first matmul
    clears the bank's has_written bits, later matmuls self-initialize their
    region (overwrite-where-unset, accumulate-where-set).
  - ScalarE applies ReLU PSUM->SBUF; out streams back as ot[i, a, j, r].
"""

import ml_dtypes
import numpy as np

import concourse.bass as bass
import concourse.mybir as mybir
import concourse.tile as tile
from concourse import bacc
from concourse.bass_utils import run_bass_kernel_spmd

B = 64          # batch (= matmul N)
CIN = 64        # in channels
COUT = 64       # out channels (= matmul M)
H = 64
W = 64
KS = 3          # conv kernel size
NCORES = 8
RPC = H // NCORES        # output rows per core = 8
NPAIR = RPC // 2         # row pairs per core = 4
NPLANES = RPC + 2        # padded input planes per core = 10
NXP = NPLANES // 2       # paired x tiles = 5
WPAD = W + 2             # 66
NJQ = 4                  # j quarter-blocks
JQ = W // NJQ            # 16 columns per block
FP32 = mybir.dt.float32
# bf16 inputs + fp32 PSUM accumulation: 4x PE throughput and half the HBM
# traffic vs fp32 (fp32 matmul lowers to 2 half-speed passes). Measured
# end-to-end max rel err ~2.5e-3.
CDT = mybir.dt.bfloat16
NP_CDT = ml_dtypes.bfloat16

_PROGRAM = None
LAST_RESULTS = None


def _build_program():
    """One Bass program, SPMD across 8 cores (inputs differ per core)."""
    nc = bacc.Bacc("TRN2", target_bir_lowering=False, debug=False,
                   num_devices=NCORES)
    # wt[t, jq, k(128), d(3), kind(3), j16, a] — see _pack_weights for k/kind.
    wt = nc.dram_tensor("wt", [NPAIR, NJQ, 128, KS, KS, JQ, COUT], CDT,
                        kind="ExternalInput")
    # xt[plane(10), b, v, r] — padded x planes for this core's rows.
    xt = nc.dram_tensor("xt", [NPLANES, CIN, WPAD, B], CDT,
                        kind="ExternalInput")
    # ot[il, a, j, r]
    ot = nc.dram_tensor("ot", [RPC, COUT, W, B], FP32, kind="ExternalOutput")

    with tile.TileContext(nc) as tc:
        with (
            tc.tile_pool(name="xpool", bufs=1) as xpool,
            tc.tile_pool(name="wpool", bufs=2) as wpool,
            tc.tile_pool(name="opool", bufs=2) as opool,
            tc.tile_pool(name="pspool", bufs=2,
                         space=bass.MemorySpace.PSUM) as pspool,
        ):
            # All x planes stay resident: 5 tiles [128=(plane parity, b), r, v].
            xp = []
            for s in range(NXP):
                # [128, v, r]: matmul rhs xp[:, v, :] streams contiguous columns
                t = xpool.tile([128, WPAD, B], CDT, tag=f"xp{s}")
                nc.sync.dma_start(
                    t[:], xt[2 * s:2 * s + 2].rearrange("p b v r -> (p b) v r"))
                xp.append(t)

            for tp in range(NPAIR):          # row pair: rows il = 2tp, 2tp+1
                for jq in range(NJQ):
                    wtile = wpool.tile([128, KS, KS, JQ, COUT], CDT, tag="w")
                    nc.sync.dma_start(wtile[:], wt[tp, jq])
                    o0 = opool.tile([COUT, JQ, B], FP32, tag="o0")
                    o1 = opool.tile([COUT, JQ, B], FP32, tag="o1")
                    for jb in range(2):      # 8-column PSUM banks
                        # Each output row accumulates in TWO banks — one per
                        # PE row-group — so all K=64 matmuls on row-group 0
                        # run concurrently with the ones on row-group 64.
                        ps0a = pspool.tile([COUT, 8, B], FP32, tag="ps0a")
                        ps0b = pspool.tile([COUT, 8, B], FP32, tag="ps0b")
                        ps1a = pspool.tile([COUT, 8, B], FP32, tag="ps1a")
                        ps1b = pspool.tile([COUT, 8, B], FP32, tag="ps1b")
                        for d in range(KS):
                            for jj in range(8):
                                jl = jb * 8 + jj          # index into wtile j16
                                j = jq * JQ + jl          # global column
                                v = j + d                 # padded x column
                                first = (d == 0 and jj == 0)
                                last = (d == KS - 1 and jj == 7)
                                # row 2tp: c=0 (plane 2tp, rows 0-63 of xp[tp])
                                nc.tensor.matmul(
                                    ps0a[:, jj, :], wtile[0:64, d, 0, jl, :],
                                    xp[tp][0:64, v, :],
                                    start=first, stop=False)
                                # row 2tp: c=1 (plane 2tp+1, rows 64-127)
                                nc.tensor.matmul(
                                    ps0b[:, jj, :], wtile[64:128, d, 0, jl, :],
                                    xp[tp][64:128, v, :],
                                    start=first, stop=last)
                                # row 2tp+1: c=1 (plane 2tp+2, rows 0-63)
                                nc.tensor.matmul(
                                    ps1a[:, jj, :], wtile[0:64, d, 1, jl, :],
                                    xp[tp + 1][0:64, v, :],
                                    start=first, stop=last)
                                # row 2tp+1: c=2 (plane 2tp+3, rows 64-127)
                                nc.tensor.matmul(
                                    ps1b[:, jj, :], wtile[64:128, d, 1, jl, :],
                                    xp[tp + 1][64:128, v, :],
                                    start=first, stop=False)
                                # row 2tp single c=2: plane 2tp+2 = upper xp[tp+1]
                                nc.tensor.matmul(
                                    ps0a[:, jj, :], wtile[0:64, d, 2, jl, :],
                                    xp[tp + 1][0:64, v, :],
                                    start=False, stop=last)
                                # row 2tp+1 single c=0: plane 2tp+1 = lower xp[tp]
                                nc.tensor.matmul(
                                    ps1b[:, jj, :], wtile[64:128, d, 2, jl, :],
                                    xp[tp][64:128, v, :],
                                    start=False, stop=last)
                        # TensorTensor may read only ONE input from PSUM:
                        # ACT copies bank a, DVE adds bank b, ACT applies ReLU.
                        ob = jb * 8
                        s0 = o0[:, ob:ob + 8, :]
                        s1 = o1[:, ob:ob + 8, :]
                        nc.scalar.activation(
                            s0, ps0a[:], mybir.ActivationFunctionType.Copy)
                        nc.scalar.activation(
                            s1, ps1a[:], mybir.ActivationFunctionType.Copy)
                        nc.vector.tensor_add(s0, s0, ps0b[:])
                        nc.vector.tensor_add(s1, s1, ps1b[:])
                        nc.scalar.activation(
                            s0, s0, mybir.ActivationFunctionType.Relu)
                        nc.scalar.activation(
                            s1, s1, mybir.ActivationFunctionType.Relu)
                    nc.sync.dma_start(ot[2 * tp, :, jq * JQ:(jq + 1) * JQ, :], o0[:])
                    nc.sync.dma_start(ot[2 * tp + 1, :, jq * JQ:(jq + 1) * JQ, :], o1[:])
    nc.compile()
    return nc


def _pack_weights(weights):
    """weights (i, j, a, b, c, d) -> WH[T, jq, k, d, kind, j16, a] per row pair.

    kind 0 (row 2T duals):   k = c*64+b, c in {0,1}
    kind 1 (row 2T+1 duals): k = (c-1)*64+b, c in {1,2}
    kind 2 (singles):        k<64: (row 2T, c=2); k>=64: (row 2T+1, c=0)
    """
    wt6 = weights.transpose(0, 5, 4, 3, 1, 2)  # [i, d, c, b, j, a]
    even = wt6[0::2]                           # [32, d, c, b, j, a]
    odd = wt6[1::2]

    def stack_k(arr):  # [32, 3(d), 2(c), 64(b), 64(j), 64(a)] -> k-major
        a = arr.transpose(0, 2, 3, 1, 4, 5)    # [32, c, b, d, j, a]
        a = a.reshape(H // 2, 128, KS, NJQ, JQ, COUT)  # j -> (jq, j16)
        return a.transpose(0, 3, 1, 2, 4, 5)   # [32, jq, k, d, j16, a]

    d0 = stack_k(even[:, :, 0:2])
    d1 = stack_k(odd[:, :, 1:3])
    s = stack_k(np.concatenate([even[:, :, 2:3], odd[:, :, 0:1]], axis=2))
    # -> [32, jq, k, d, kind, j16, a]
    return np.ascontiguousarray(np.stack([d0, d1, s], axis=4))


def _prep_x(x):
    xpad = np.pad(x, ((0, 0), (0, 0), (1, 1), (1, 1)))
    return np.ascontiguousarray(xpad.transpose(2, 1, 3, 0))  # [u, b, v, r]


def kernel(x, weights):
    global _PROGRAM, LAST_RESULTS
    x = np.ascontiguousarray(np.asarray(x, dtype=np.float32))
    weights = np.ascontiguousarray(np.asarray(weights, dtype=np.float32))
    assert x.shape == (B, CIN, H, W) and weights.shape == (H, W, COUT, CIN, KS, KS)

    x_t = _prep_x(x)
    wh = _pack_weights(weights)                             # [32, jq, k, d, e, j16, a]

    wh = wh.astype(NP_CDT)
    x_t = x_t.astype(NP_CDT)
    in_maps = []
    for cid in range(NCORES):
        in_maps.append({
            "wt": np.ascontiguousarray(wh[4 * cid:4 * cid + 4]),
            "xt": np.ascontiguousarray(x_t[RPC * cid:RPC * cid + NPLANES]),
        })

    if _PROGRAM is None:
        _PROGRAM = _build_program()
    res = run_bass_kernel_spmd(_PROGRAM, in_maps, list(range(NCORES)))
    LAST_RESULTS = res

    # ot[il, a, j, r] per core -> out[r, a, i, j]
    full = np.concatenate([res.results[c]["ot"] for c in range(NCORES)], axis=0)
    return np.ascontiguousarray(full.transpose(3, 1, 0, 2))



# revision 6
# speedup vs baseline: 1.1617x; 1.1617x over previous
"""Locally-connected conv (per-location weights) + ReLU on 8 Trainium2 cores.

Problem: x (B=64, Cin=64, H=64, W=64), weights (H, W, Cout=64, Cin=64, 3, 3)
  out[r,a,i,j] = relu( sum_{b,c,d} weights[i,j,a,b,c,d] * xpad[r,b,i+c,j+d] )

Sharding: data-parallel over H — core cid owns output rows i in [8*cid, 8*cid+8).
No collectives; pure SPMD with per-core input slices.

Device strategy (per core), v2 — dense K=128/M=128 duals:
  - x planes are packed pairwise into 128-partition tiles A_s = (plane 2s,
    2s+1) x Cin, resident in SBUF for the whole kernel.
  - Vertical-tap pairing: output row pair (2s-1, 2s) consumes tile A_s with
    both planes valid, so one K=128 x M=128 matmul per (j, d) accumulates
    BOTH rows' dual taps (even rows c=0,1; odd rows c=1,2) with a fully
    dense stationary — no zero padding, full PE utilization.
  - Leftover single taps (even rows c=2, odd rows c=0) are K=64 matmuls on
    opposite partition halves / opposite PSUM column groups, so pairs run
    concurrently on disjoint 32x32 PE sub-arrays.
  - Boundary rows 0 and 7 get M=64 duals sharing one PSUM bank.
  - Each output location accumulates its 9 taps in ONE PSUM bank; a single
    ScalarE ReLU (fp32 PSUM -> bf16 SBUF) finishes it — no DVE adds.
  - Weights stream as 8 chunks of 4.7 MB (36 KB contiguous per partition
    line), triple-buffered so the DMA queue never starves; output leaves in
    two 2 MB bf16 transfers (host upcasts to fp32).
"""

import ml_dtypes
import numpy as np

import concourse.bass as bass
import concourse.mybir as mybir
import concourse.tile as tile
from concourse import bacc
from concourse.bass_utils import run_bass_kernel_spmd

B = 64          # batch (= matmul N)
CIN = 64        # in channels
COUT = 64       # out channels
H = 64
W = 64
KS = 3          # conv kernel size
NCORES = 8
RPC = H // NCORES        # output rows per core = 8
NPLANES = RPC + 2        # padded input planes per core = 10
WPAD = W + 2             # 66
NJB = 8                  # j-blocks of 8 columns
CHUNK = 18432            # weight elements per partition per j-block
FP32 = mybir.dt.float32
# bf16 inputs + fp32 PSUM accumulation; bf16 output (host upcasts).
CDT = mybir.dt.bfloat16
NP_CDT = ml_dtypes.bfloat16

_PROGRAM = None
LAST_RESULTS = None


def _d_off(s, jl, d):
    return (((s - 1) * 8 + jl) * 3 + d) * 128


def _b_off(e, jl, d):
    return 9216 + ((e * 8 + jl) * 3 + d) * 64


def _s_off(ri, jl, d):
    return 12288 + ((ri * 8 + jl) * 3 + d) * 64


def _build_program():
    """One Bass program, SPMD across 8 cores (inputs differ per core)."""
    nc = bacc.Bacc("TRN2", target_bir_lowering=False, debug=False,
                   num_devices=NCORES)
    # wt[jb, k(128), CHUNK] — see _pack_weights for the free-dim layout.
    wt = nc.dram_tensor("wt", [NJB, 128, CHUNK], CDT, kind="ExternalInput")
    # xt[plane(10), b, v, r] — padded x planes for this core's rows.
    xt = nc.dram_tensor("xt", [NPLANES, CIN, WPAD, B], CDT,
                        kind="ExternalInput")
    # ot[part(128), half, bank, jb4, jl, r]
    ot = nc.dram_tensor("ot", [128, 2, 4, 4, 8, B], CDT, kind="ExternalOutput")

    with tile.TileContext(nc) as tc:
        with (
            tc.tile_pool(name="xpool", bufs=1) as xpool,
            tc.tile_pool(name="opool", bufs=1) as opool,
            tc.tile_pool(name="wpool", bufs=3) as wpool,
            tc.tile_pool(name="pspool", bufs=2,
                         space=bass.MemorySpace.PSUM) as pspool,
        ):
            # All x planes stay resident: 5 tiles [128=(plane parity, b), v, r].
            A = []
            for s in range(5):
                t = xpool.tile([128, WPAD, B], CDT, tag=f"xp{s}")
                nc.sync.dma_start(
                    t[:], xt[2 * s:2 * s + 2].rearrange("p b v r -> (p b) v r"))
                A.append(t)
            out_sb = opool.tile([128, 2, 4, 4, 8, B], CDT, tag="out")

            for jb in range(NJB):
                wtile = wpool.tile([128, CHUNK], CDT, tag="w")
                nc.sync.dma_start(wtile[:], wt[jb])
                ps = [pspool.tile([128, 8, B], FP32, tag=f"ps{k}",
                                  name=f"ps{k}")
                      for k in range(4)]

                # PSUM has_written clears are partition-masked: start=True
                # only clears the partitions the matmul writes. Track first/
                # last writer per (bank, partition half) so each half of each
                # bank gets exactly one clearing start and one stop.
                calls = []   # (bank, halves_mask, out, lhsT, rhs)
                for jl in range(8):
                    for d in range(KS):
                        v = 8 * jb + jl + d
                        for s in (1, 2, 3):
                            o = _d_off(s, jl, d)
                            calls.append((s, 3, ps[s][:, jl, :],
                                          wtile[:, o:o + 128], A[s][:, v, :]))
                        o = _b_off(0, jl, d)       # row 0 duals -> cols 64-127
                        calls.append((0, 2, ps[0][64:128, jl, :],
                                      wtile[:, o:o + 64], A[0][:, v, :]))
                        o = _b_off(1, jl, d)       # row 7 duals -> cols 0-63
                        calls.append((0, 1, ps[0][0:64, jl, :],
                                      wtile[:, o:o + 64], A[4][:, v, :]))
                        for il in (0, 2, 4, 6):    # even singles: c=2
                            ri = il // 2
                            bank = 0 if il == 0 else il // 2
                            sx = il // 2 + 1
                            o = _s_off(ri, jl, d)
                            calls.append((bank, 2, ps[bank][64:128, jl, :],
                                          wtile[0:64, o:o + 64],
                                          A[sx][0:64, v, :]))
                        for il in (1, 3, 5, 7):    # odd singles: c=0
                            ri = il // 2
                            bank = 0 if il == 7 else (il + 1) // 2
                            sx = (il - 1) // 2
                            o = _s_off(ri, jl, d)
                            calls.append((bank, 1, ps[bank][0:64, jl, :],
                                          wtile[64:128, o:o + 64],
                                          A[sx][64:128, v, :]))

                first, last = {}, {}
                for idx, (bank, halves, _, _, _) in enumerate(calls):
                    for h in (1, 2):
                        if halves & h:
                            first.setdefault((bank, h), idx)
                            last[(bank, h)] = idx
                firsts, lasts = set(first.values()), set(last.values())
                for idx, (bank, halves, o_ap, l_ap, r_ap) in enumerate(calls):
                    nc.tensor.matmul(o_ap, l_ap, r_ap,
                                     start=(idx in firsts),
                                     stop=(idx in lasts))

                for k in range(4):
                    nc.scalar.activation(
                        out_sb[:, jb // 4, k, jb % 4, :, :], ps[k][:],
                        mybir.ActivationFunctionType.Relu)
                if jb == 3:
                    nc.sync.dma_start(ot[:, 0], out_sb[:, 0])
                if jb == 7:
                    nc.sync.dma_start(ot[:, 1], out_sb[:, 1])
    nc.compile()
    return nc


def _pack_weights(w):
    """weights slice (il 8, j, a, b, c, d) for one core -> [NJB, 128, CHUNK].

    Free-dim layout per partition line (k = vertical-tap parity * 64 + b for
    duals, k = single-parity-specific):
      D [s(3), jl(8), d(3), m(128)]   dual rows (2s-1, 2s); m = half*64 + a
      B [e(2), jl(8), d(3), m(64)]    e=0 row 0 (c=ph), e=1 row 7 (c=ph+1)
      S [ri(4), jl(8), d(3), a(64)]   parts 0-63: even rows c=2;
                                      parts 64-127: odd rows c=0
    """
    lo = w[[1, 3, 5]][:, :, :, :, 1:3, :]   # rows 2s-1, c=ph+1: [s,j,a,b,ph,d]
    hi = w[[2, 4, 6]][:, :, :, :, 0:2, :]   # rows 2s,   c=ph

    def dpart(arr):  # [s, jg, a, b, ph, d] -> [jb, ph, b, s, jl, d, a]
        t = arr.transpose(4, 3, 0, 1, 5, 2)
        t = t.reshape(2, 64, 3, 8, 8, 3, 64)
        return t.transpose(3, 0, 1, 2, 4, 5, 6)

    D = np.stack([dpart(lo), dpart(hi)], axis=6)   # [jb,ph,b,s,jl,d,half,a]
    D = D.reshape(NJB, 128, 9216)

    r0 = w[0][:, :, :, 0:2, :]   # [jg, a, b, ph, d], c=ph
    r7 = w[7][:, :, :, 1:3, :]   # c=ph+1

    def bpart(arr):  # [jg, a, b, ph, d] -> [jb, ph, b, jl, d, a]
        t = arr.transpose(3, 2, 0, 4, 1)
        t = t.reshape(2, 64, 8, 8, 3, 64)
        return t.transpose(2, 0, 1, 3, 4, 5)

    Bv = np.stack([bpart(r0), bpart(r7)], axis=3)  # [jb, ph, b, e, jl, d, a]
    Bv = Bv.reshape(NJB, 128, 3072)

    ev = w[[0, 2, 4, 6]][:, :, :, :, 2, :]   # [row, jg, a, b, d]
    od = w[[1, 3, 5, 7]][:, :, :, :, 0, :]

    def spart(arr):  # [row, jg, a, b, d] -> [jb, b, row, jl, d, a]
        t = arr.transpose(3, 0, 1, 4, 2)
        t = t.reshape(64, 4, 8, 8, 3, 64)
        return t.transpose(2, 0, 1, 3, 4, 5)

    S = np.concatenate([spart(ev), spart(od)], axis=1)  # [jb,128,row,jl,d,a]
    S = S.reshape(NJB, 128, 6144)

    return np.concatenate([D, Bv, S], axis=2)  # [NJB, 128, CHUNK]


def _prep_x(x):
    xpad = np.pad(x, ((0, 0), (0, 0), (1, 1), (1, 1)))
    return np.ascontiguousarray(xpad.transpose(2, 1, 3, 0))  # [u, b, v, r]


_ROWS_LO = (7, 1, 3, 5)   # PSUM parts 0-63 by bank
_ROWS_HI = (0, 2, 4, 6)   # PSUM parts 64-127 by bank


def _unpack_out(ot_core):
    """ot [128, 2, 4, 4, 8, 64] bf16 -> [r, a, il, j] fp32 for one core."""
    view = np.asarray(ot_core, dtype=np.float32).reshape(2, 64, 2, 4, 4, 8, B)
    res = np.empty((B, COUT, RPC, W), np.float32)
    for bank in range(4):
        for ph, row in ((0, _ROWS_LO[bank]), (1, _ROWS_HI[bank])):
            arr = view[ph, :, :, bank]                    # [a, half, jb4, jl, r]
            res[:, :, row, :] = arr.transpose(4, 0, 1, 2, 3).reshape(B, COUT, W)
    return res


def kernel(x, weights):
    global _PROGRAM, LAST_RESULTS
    x = np.ascontiguousarray(np.asarray(x, dtype=np.float32))
    weights = np.ascontiguousarray(np.asarray(weights, dtype=np.float32))
    assert x.shape == (B, CIN, H, W) and weights.shape == (H, W, COUT, CIN, KS, KS)

    x_t = _prep_x(x).astype(NP_CDT)

    in_maps = []
    for cid in range(NCORES):
        wh = _pack_weights(weights[RPC * cid:RPC * cid + RPC]).astype(NP_CDT)
        in_maps.append({
            "wt": np.ascontiguousarray(wh),
            "xt": np.ascontiguousarray(x_t[RPC * cid:RPC * cid + NPLANES]),
        })

    if _PROGRAM is None:
        _PROGRAM = _build_program()
    res = run_bass_kernel_spmd(_PROGRAM, in_maps, list(range(NCORES)))
    LAST_RESULTS = res

    full = np.empty((B, COUT, H, W), np.float32)
    for cid in range(NCORES):
        full[:, :, RPC * cid:RPC * cid + RPC, :] = _unpack_out(
            res.results[cid]["ot"])
    return full
